# revision 11
# baseline (speedup 1.0000x reference)
"""Trainium2 Bass kernel for nn_Encoder (skip-gram style encoder).

Computation (per batch element b, B=16384, K=20, D=128, H=256, L=128):
    center_e  = emb[center_ids[b]]                    # [D]
    context_e = emb[context_ids[b, k]]                # [K, D]
    h   = sum_k relu(center_e @ Wf[:D] + context_e_k @ Wf[D:] + bf)   # [H]
    mu  = h @ Wu + bu                                 # [L]
    sigma = exp(h @ Wv + bv)                          # [1]

Sharding: data-parallel over the batch dim across 8 NeuronCores (2048 rows
each); the 51 MB embedding table and the small MLP weights are replicated.

Device strategy per 128-row tile (16 tiles/core):
  1. One SWDGE indirect-DMA gathers all 21 embedding rows per batch row
     (center + 20 context) into SBUF in natural [row, slot, D] layout
     (2688 descriptors x 512 B).
  2. PE transposes each [128,128] slot into PSUM (identity matmul);
     ScalarE evicts 4 slots at a time to SBUF -> X_T in [D, row] layout.
  3. Per hidden-half (2) and context slot k (20): PE matmul accumulates
     Wf_x_half.T @ X_T[k] + Wf_c_half.T @ X_T[center] into PSUM ([H/2, rows]).
  4. ScalarE applies relu with per-partition bias bf while evicting PSUM.
  5. DVE reduces over k (strided AP, X-axis reduce) -> h half.
  6. PE computes mu (h @ Wu, + bu via a K=1 ones-matmul) and h @ Wv in PSUM;
     ScalarE evicts mu and applies exp(.+bv) for sigma; DMA out.
"""

import numpy as np

import concourse.bass as bass
import concourse.mybir as mybir
import concourse.tile as tile
from concourse import bacc
from concourse.bass import IndirectOffsetOnAxis
from concourse.bass_utils import run_bass_kernel_spmd
from concourse.masks import make_identity

VOCAB = 100000
D = 128
H = 256
L = 128
B = 16384
K = 20
NCORES = 8
BS = B // NCORES          # rows per core
P = 128                   # partition / tile row count
NT = BS // P              # tiles per core
NSLOT = 1 + K             # center + context slots per row

F32 = mybir.dt.float32
I32 = mybir.dt.int32

_CACHE = {}


def _build_program(reps=1):
    nc = bacc.Bacc()

    emb_d = nc.dram_tensor("emb_t", [VOCAB, D], F32, kind="ExternalInput")
    ids_d = nc.dram_tensor("ids_t", [P, NT * NSLOT], I32, kind="ExternalInput")
    wfc_d = nc.dram_tensor("wfc_t", [D, H], F32, kind="ExternalInput")
    wfx_d = nc.dram_tensor("wfx_t", [D, H], F32, kind="ExternalInput")
    wu_d = nc.dram_tensor("wu_t", [H // 2, 2 * L], F32, kind="ExternalInput")
    wv_d = nc.dram_tensor("wv_t", [H // 2, 2], F32, kind="ExternalInput")
    bf_d = nc.dram_tensor("bf_t", [P, 2], F32, kind="ExternalInput")
    bu_d = nc.dram_tensor("bu_t", [1, L], F32, kind="ExternalInput")
    bv_d = nc.dram_tensor("bv_t", [P, 1], F32, kind="ExternalInput")

    mu_d = nc.dram_tensor("mu_t", [BS, L], F32, kind="ExternalOutput")
    sg_d = nc.dram_tensor("sg_t", [BS, 1], F32, kind="ExternalOutput")

    with tile.TileContext(nc) as tc:
        with (
            tc.tile_pool(name="const", bufs=1) as cpool,
            tc.tile_pool(name="xnat", bufs=2) as xnat_pool,
            tc.tile_pool(name="xt", bufs=2) as xt_pool,
            tc.tile_pool(name="rbuf", bufs=2) as r_pool,
            tc.tile_pool(name="hbuf", bufs=2) as h_pool,
            tc.tile_pool(name="osb", bufs=2) as o_pool,
            tc.tile_pool(name="pst", bufs=2, space="PSUM") as pt_pool,
            tc.tile_pool(name="psz", bufs=1, space="PSUM") as pz_pool,
            tc.tile_pool(name="pso", bufs=1, space="PSUM") as po_pool,
        ):
            idt = cpool.tile([P, P], F32)
            make_identity(nc, idt[:, :])

            ids_sb = cpool.tile([P, NT * NSLOT], I32)
            nc.sync.dma_start(out=ids_sb[:, :], in_=ids_d[:, :])
            # weights: wfc/wfx halves [D, 128] each
            wfc_sb = cpool.tile([D, H], F32)
            nc.sync.dma_start(out=wfc_sb[:, :], in_=wfc_d[:, :])
            wfx_sb = cpool.tile([D, H], F32)
            nc.sync.dma_start(out=wfx_sb[:, :], in_=wfx_d[:, :])
            wu_sb = cpool.tile([H // 2, 2 * L], F32)  # half h at [:, h*L:(h+1)*L]
            nc.sync.dma_start(out=wu_sb[:, :], in_=wu_d[:, :])
            wv_sb = cpool.tile([H // 2, 2], F32)
            nc.sync.dma_start(out=wv_sb[:, :], in_=wv_d[:, :])
            bf_sb = cpool.tile([P, 2], F32)
            nc.sync.dma_start(out=bf_sb[:, :], in_=bf_d[:, :])
            bu_sb = cpool.tile([1, L], F32, padded_shape=[P, L])
            nc.sync.dma_start(out=bu_sb[:1, :], in_=bu_d[:, :])
            bv_sb = cpool.tile([P, 1], F32)
            nc.sync.dma_start(out=bv_sb[:, :], in_=bv_d[:, :])
            ones_sb = cpool.tile([1, P], F32, padded_shape=[P, P])
            nc.vector.memset(ones_sb[:1, :], 1.0)

            for t in [tt for _ in range(reps) for tt in range(NT)]:
                # ---- 1. gather the 21*128 embedding rows for this tile ----
                # HW indirect DMA consumes ONE index per dest partition, so
                # gather one slot-column (128 rows) per instruction.
                xnat = xnat_pool.tile([P, NSLOT * D], F32, tag="xnat")
                for s in range(NSLOT):
                    nc.gpsimd.indirect_dma_start(
                        out=xnat[:, s * D : (s + 1) * D],
                        out_offset=None,
                        in_=emb_d[:, :],
                        in_offset=IndirectOffsetOnAxis(
                            ap=ids_sb[:, t * NSLOT + s : t * NSLOT + s + 1], axis=0
                        ),
                    )

                # ---- 2. transpose each slot: [row, d] -> [d, row] ----
                xt = xt_pool.tile([P, NSLOT * D], F32, tag="xt")
                for g in range(6):  # groups of 4 slots (last group has 1)
                    n = 4 if g < 5 else 1
                    ps = pt_pool.tile([P, 4 * P], F32, tag="pt")
                    for j in range(n):
                        s = g * 4 + j
                        nc.tensor.transpose(
                            out=ps[:, j * P : (j + 1) * P],
                            in_=xnat[:, s * D : (s + 1) * D],
                            identity=idt[:, :],
                        )
                    nc.scalar.copy(
                        out=xt[:, g * 4 * P : g * 4 * P + n * P], in_=ps[:, : n * P]
                    )

                h_sb = h_pool.tile([P, H], F32, tag="h")
                for hh in range(2):
                    # ---- 3. z_k = Wf_x_h.T @ x_k + Wf_c_h.T @ x_center ----
                    zt = pz_pool.tile([P, K * P], F32, tag="zt")
                    for k in range(K):
                        nc.tensor.matmul(
                            out=zt[:, k * P : (k + 1) * P],
                            lhsT=wfx_sb[:, hh * P : (hh + 1) * P],
                            rhs=xt[:, (1 + k) * D : (2 + k) * D],
                            start=True,
                            stop=False,
                        )
                        nc.tensor.matmul(
                            out=zt[:, k * P : (k + 1) * P],
                            lhsT=wfc_sb[:, hh * P : (hh + 1) * P],
                            rhs=xt[:, 0:D],
                            start=False,
                            stop=True,
                        )
                    # ---- 4. relu(z + bf) eviction, 4 slots per ACT op ----
                    rt = r_pool.tile([P, K * P], F32, tag="r")
                    for gb in range(5):
                        nc.scalar.activation(
                            out=rt[:, gb * 4 * P : (gb + 1) * 4 * P],
                            in_=zt[:, gb * 4 * P : (gb + 1) * 4 * P],
                            func=mybir.ActivationFunctionType.Relu,
                            bias=bf_sb[:, hh : hh + 1],
                            scale=1.0,
                        )
                    # ---- 5. h_half = sum_k r_k  (X-axis reduce over k) ----
                    nc.vector.tensor_reduce(
                        out=h_sb[:, hh * P : (hh + 1) * P],
                        in_=rt[:, :].rearrange("p (k r) -> p r k", k=K),
                        axis=mybir.AxisListType.X,
                        op=mybir.AluOpType.add,
                    )

                # ---- 6. mu = h @ Wu + bu ; sigma = exp(h @ Wv + bv) ----
                mo = po_pool.tile([P, L + 4], F32, tag="mo")
                for hh in range(2):
                    nc.tensor.matmul(
                        out=mo[:, 0:L],
                        lhsT=h_sb[:, hh * P : (hh + 1) * P],
                        rhs=wu_sb[:, hh * L : (hh + 1) * L],
                        start=(hh == 0),
                        stop=False,
                    )
                nc.tensor.matmul(
                    out=mo[:, 0:L],
                    lhsT=ones_sb[:1, :],
                    rhs=bu_sb[:1, :],
                    start=False,
                    stop=True,
                )
                for hh in range(2):
                    nc.tensor.matmul(
                        out=mo[:, L : L + 1],
                        lhsT=h_sb[:, hh * P : (hh + 1) * P],
                        rhs=wv_sb[:, hh : hh + 1],
                        start=(hh == 0),
                        stop=(hh == 1),
                    )
                ot = o_pool.tile([P, L + 1], F32, tag="ot")
                nc.scalar.copy(out=ot[:, 0:L], in_=mo[:, 0:L])
                nc.scalar.activation(
                    out=ot[:, L : L + 1],
                    in_=mo[:, L : L + 1],
                    func=mybir.ActivationFunctionType.Exp,
                    bias=bv_sb[:, 0:1],
                    scale=1.0,
                )
                nc.sync.dma_start(
                    out=mu_d[t * P : (t + 1) * P, :], in_=ot[:, 0:L]
                )
                nc.sync.dma_start(
                    out=sg_d[t * P : (t + 1) * P, :], in_=ot[:, L : L + 1]
                )

    nc.finalize()
    return nc


def _get_program():
    if "nc" not in _CACHE:
        _CACHE["nc"] = _build_program()
    return _CACHE["nc"]


def kernel(center_ids, context_ids, emb, Wf, bf, Wu, bu, Wv, bv, _trace=False):
    center_ids = np.asarray(center_ids).astype(np.int32)
    context_ids = np.asarray(context_ids).astype(np.int32)
    emb = np.ascontiguousarray(np.asarray(emb, dtype=np.float32))
    Wf = np.asarray(Wf, dtype=np.float32)
    bf = np.asarray(bf, dtype=np.float32)
    Wu = np.ascontiguousarray(np.asarray(Wu, dtype=np.float32))
    bu = np.asarray(bu, dtype=np.float32)
    Wv = np.ascontiguousarray(np.asarray(Wv, dtype=np.float32))
    bv = np.asarray(bv, dtype=np.float32)

    # ids packed per core: ids_sb[p, t*NSLOT + s]  (s=0 center, 1..K context)
    ids_all = np.concatenate([center_ids[:, None], context_ids], axis=1)  # [B, 21]
    ids_all = ids_all.reshape(NCORES, NT, P, NSLOT)

    wfc = np.ascontiguousarray(Wf[:D, :])   # [D, H]
    wfx = np.ascontiguousarray(Wf[D:, :])   # [D, H]
    wu_packed = np.ascontiguousarray(np.concatenate([Wu[: H // 2], Wu[H // 2 :]], axis=1))
    wv_packed = np.ascontiguousarray(np.concatenate([Wv[: H // 2], Wv[H // 2 :]], axis=1))
    bf_packed = np.ascontiguousarray(bf.reshape(2, P).T)  # [P, 2]
    bu_packed = np.ascontiguousarray(bu.reshape(1, L))
    bv_packed = np.full((P, 1), np.float32(bv[0]), dtype=np.float32)

    in_maps = []
    for c in range(NCORES):
        ids_c = np.ascontiguousarray(
            ids_all[c].transpose(1, 0, 2).reshape(P, NT * NSLOT)
        )
        in_maps.append(
            {
                "emb_t": emb,
                "ids_t": ids_c,
                "wfc_t": wfc,
                "wfx_t": wfx,
                "wu_t": wu_packed,
                "wv_t": wv_packed,
                "bf_t": bf_packed,
                "bu_t": bu_packed,
                "bv_t": bv_packed,
            }
        )

    nc = _get_program()
    res = run_bass_kernel_spmd(
        nc, in_maps, core_ids=list(range(NCORES)), trace=_trace
    )
    mu = np.concatenate([r["mu_t"] for r in res.results], axis=0)
    sigma = np.concatenate([r["sg_t"] for r in res.results], axis=0)
    if _trace:
        _CACHE["last_results"] = res
    return mu, sigma
